# revision 1
# baseline (speedup 1.0000x reference)
"""Bass/Tile TRN2 kernel for nn_CropDrones.

Op: per-sample, find bbox of a binary window mask (channel 3 of input1),
crop rows [r0, r1) x cols [c0, c1) of the 3 image channels, and paste the
crop centered into a 256x256 zero canvas.

Sharding: pure data parallel - batch 32 split as 8 cores x 4 samples.

Device strategy (per sample, all control flow static; data dependence
flows only through values):
  1. Load the 512x512 mask as [128p, 4n, 512w]; reduce to row-any [128,4]
     (free-dim max) and col-any [128,512] (two contiguous pairwise maxes).
  2. bbox scalars via the weighted-max trick: r1 = max(r * any_r),
     511 - r0 = max((511-r) * any_r) (same for cols), finished with a
     gpsimd partition_all_reduce so every partition holds all 4 scalars.
  3. f32 arithmetic + int32 shift-right (exact floor-div) computes
     top/left indents, row shift rt = r0 - top, column shift d = c0-left.
  4. Row gather: 6 indirect DMAs (one per 128-row output slot), each with
     a per-partition index table [128,1]:
     idx(p,j) = b*2048 + (j//2)*512 + clip(rt + 128*(j%2) + p, 0, 511),
     pulling full 512-float source rows so output row y lands directly at
     partition y%128, slot 2*(c) + y//128.
  5. gpsimd ap_gather applies the fine column shift d (clamped; positions
     outside the window read garbage that the masks zero).
  6. Multiply by valid_y (per-partition scalar) x valid_x (row vector)
     0/1 masks via one scalar_tensor_tensor per slot; DMA [128,6,256] out.

Verified bit-exact vs the jax reference (relative error 0.0). Cost-model
(TimelineSim) per-core time ~85us; HBM traffic ~12.6MB/core.
"""

import numpy as np

import concourse.bass as bass
import concourse.bacc as bacc
import concourse.bass_isa as bass_isa
import concourse.mybir as mybir
from concourse.bass import IndirectOffsetOnAxis
from concourse.bass_utils import run_bass_kernel_spmd
from concourse.tile import TileContext

# Problem shapes (hardcoded; kernel.py must be self-contained).
B, C, H, W = 32, 3, 512, 512
CH_IN = 4          # image channels + mask channel
S = 256            # output side
N_CORES = 8
BPC = B // N_CORES  # samples per core
P = 128
NT = H // P        # mask row tiles
NJ = C * (S // P)  # gather slots (channel-major: j = 2c + y//128)
GW = 512           # gathered window width (floats; full image rows)

f32 = mybir.dt.float32
i32 = mybir.dt.int32
i16 = mybir.dt.int16
Alu = mybir.AluOpType
Ax = mybir.AxisListType


_C_WIDTHS = {
    "c_iota_w": W,
    "c_rev_w": W,
    "c_iota_r": NT,
    "c_rev_r": NT,
    "c_y_tab": NJ,
    "c_c_tab": NJ,
    "c_jx_j": (NJ * S) // 16,
    "c_jx_x": (NJ * S) // 16,
    "c_g_y": 48,
    "c_g_c": 48,
}
_C_OFFS = {}
_off = 0
for _k, _w in _C_WIDTHS.items():
    _C_OFFS[_k] = _off
    _off += _w
C_TOTAL = _off


def _consts() -> dict[str, np.ndarray]:
    p = np.arange(P)
    iota_w = np.broadcast_to(np.arange(W, dtype=np.float32), (P, W)).copy()
    rev_w = (W - 1.0) - iota_w
    iota_r = (p[:, None] + P * np.arange(NT)[None, :]).astype(np.float32)
    rev_r = (H - 1.0) - iota_r
    j = np.arange(NJ)
    y_tab = (P * (j[None, :] % 2) + p[:, None]).astype(np.float32)
    c_tab = np.broadcast_to((W * (j // 2)).astype(np.float32), (P, NJ)).copy()
    # ap_gather index tables, wrapped in groups of 16 partitions:
    # flat gather position k = s*16 + (p % 16); k -> (slot j, column x).
    s_ = np.arange((NJ * S) // 16)
    k = s_[None, :] * 16 + (p[:, None] % 16)
    jx_j = ((k // S) * GW).astype(np.float32)
    jx_x = (k % S).astype(np.float32)
    # row-gather index tables, wrapped: kk = s*16 + p%16 in [0, 768) -> (c, y)
    s2 = np.arange(48)
    kk = s2[None, :] * 16 + (p[:, None] % 16)
    g_y = (kk % (2 * P)).astype(np.float32)
    g_c = ((kk // (2 * P)) * W).astype(np.float32)
    vals = {
        "c_iota_w": iota_w,
        "c_rev_w": rev_w,
        "c_iota_r": iota_r,
        "c_rev_r": rev_r,
        "c_y_tab": y_tab,
        "c_c_tab": c_tab,
        "c_jx_j": jx_j,
        "c_jx_x": jx_x,
        "c_g_y": g_y,
        "c_g_c": g_c,
    }
    packed = np.zeros((P, C_TOTAL), dtype=np.float32)
    for kk, vv in vals.items():
        packed[:, _C_OFFS[kk] : _C_OFFS[kk] + _C_WIDTHS[kk]] = vv
    return {"c_all": packed}


DEBUG_TAPS = False
GATHER_MODE = "indirect6"  # "dma_gather" hit an opaque runtime failure on HW
SKIP = set()  # sim-bisect only: 'gather','apg','colred','par','fin','maskload'


def _build() -> bass.Bass:
    nc = bacc.Bacc("TRN2")
    x = nc.dram_tensor("x", [BPC, CH_IN, H, W], f32, kind="ExternalInput")
    y = nc.dram_tensor("y", [BPC, C, S, S], f32, kind="ExternalOutput")
    c_all = nc.dram_tensor("c_all", [P, C_TOTAL], f32, kind="ExternalInput")
    if DEBUG_TAPS:
        d_ar = nc.dram_tensor("d_ar", [BPC, P, 4], f32, kind="ExternalOutput")
        d_sc = nc.dram_tensor("d_sc", [BPC, P, 13], f32, kind="ExternalOutput")
        d_ri = nc.dram_tensor("d_ri", [BPC, P, NJ], i32, kind="ExternalOutput")
        d_ci = nc.dram_tensor("d_ci", [BPC, P, (NJ * S) // 16], i16, kind="ExternalOutput")
        d_gat = nc.dram_tensor("d_gat", [BPC, P, NJ, GW], f32, kind="ExternalOutput")
        d_ext = nc.dram_tensor("d_ext", [BPC, P, NJ * S], f32, kind="ExternalOutput")

    with TileContext(nc) as tc:
        with (
            tc.tile_pool(name="consts", bufs=1) as cpool,
            tc.tile_pool(name="work", bufs=4) as wp,
        ):
            call_t = cpool.tile([P, C_TOTAL], f32, tag="c_all", name="c_all_t")
            nc.sync.dma_start(call_t[:], c_all[:])
            ct = {
                k: call_t[:, _C_OFFS[k] : _C_OFFS[k] + _C_WIDTHS[k]]
                for k in _C_WIDTHS
            }

            ts = nc.vector.tensor_scalar
            st = nc.vector.scalar_tensor_tensor
            tt = nc.vector.tensor_tensor
            red = nc.vector.tensor_reduce
            cpy = nc.vector.tensor_copy

            for i in range(BPC):
                # ---- 1. mask load + reductions ----
                mt = wp.tile([P, NT, W], f32, tag="mt")
                if "maskload" not in SKIP:
                    nc.sync.dma_start(
                        mt[:], x[i, CH_IN - 1].rearrange("(n p) w -> p n w", p=P)
                    )
                rowany = wp.tile([P, NT], f32, tag="rowany")
                red(rowany[:], mt[:], axis=Ax.X, op=Alu.max)
                colp = wp.tile([P, W], f32, tag="colp")
                tp2 = wp.tile([P, 2, W], f32, tag="tp2")
                tt(tp2[:], mt[:, 0:2, :], mt[:, 2:4, :], op=Alu.max)
                tt(colp[:], tp2[:, 0, :], tp2[:, 1, :], op=Alu.max)

                # ---- 2. bbox scalars ----
                cand = wp.tile([P, 4], f32, tag="cand")
                t4 = wp.tile([P, NT], f32, tag="t4")
                t4b = wp.tile([P, NT], f32, tag="t4b")
                tw = wp.tile([P, W], f32, tag="tw")
                twb = wp.tile([P, W], f32, tag="twb")
                tt(t4[:], rowany[:], ct["c_iota_r"], op=Alu.mult)
                red(cand[:, 0:1], t4[:], axis=Ax.X, op=Alu.max)  # r1
                tt(t4b[:], rowany[:], ct["c_rev_r"], op=Alu.mult)
                red(cand[:, 1:2], t4b[:], axis=Ax.X, op=Alu.max)  # 511 - r0
                tt(tw[:], colp[:], ct["c_iota_w"], op=Alu.mult)
                red(cand[:, 2:3], tw[:], axis=Ax.X, op=Alu.max)  # c1
                tt(twb[:], colp[:], ct["c_rev_w"], op=Alu.mult)
                red(cand[:, 3:4], twb[:], axis=Ax.X, op=Alu.max)  # 511 - c0
                ar = wp.tile([P, 4], f32, tag="ar")
                if "par" not in SKIP:
                    nc.gpsimd.partition_all_reduce(
                        ar[:], cand[:], channels=P, reduce_op=bass_isa.ReduceOp.max
                    )
                else:
                    cpy(ar[:], cand[:])

                # ---- 3. derived scalars ----
                # sc columns: 0 sh, 1 sw, 2 top, 3 left, 4 rt, 5 d, 6 q, 7 e,
                #             8 tph, 9 lw, 10 r0, 11 c0, 12 tmp
                sc = wp.tile([P, 13], f32, tag="sc")
                sci = wp.tile([P, 3], i32, tag="sci")
                st(sc[:, 0:1], ar[:, 0:1], -511.0, ar[:, 1:2], op0=Alu.add, op1=Alu.add)
                st(sc[:, 1:2], ar[:, 2:3], -511.0, ar[:, 3:4], op0=Alu.add, op1=Alu.add)
                ts(sc[:, 12:13], sc[:, 0:1], -1.0, 256.0, op0=Alu.mult, op1=Alu.add)
                cpy(sci[:, 0:1], sc[:, 12:13])
                ts(sci[:, 0:1], sci[:, 0:1], 1, None, op0=Alu.logical_shift_right)
                cpy(sc[:, 2:3], sci[:, 0:1])  # top = (256 - sh) >> 1
                ts(sc[:, 12:13], sc[:, 1:2], -1.0, 256.0, op0=Alu.mult, op1=Alu.add)
                cpy(sci[:, 1:2], sc[:, 12:13])
                ts(sci[:, 1:2], sci[:, 1:2], 1, None, op0=Alu.logical_shift_right)
                cpy(sc[:, 3:4], sci[:, 1:2])  # left = (256 - sw) >> 1
                ts(sc[:, 10:11], ar[:, 1:2], -1.0, 511.0, op0=Alu.mult, op1=Alu.add)
                ts(sc[:, 11:12], ar[:, 3:4], -1.0, 511.0, op0=Alu.mult, op1=Alu.add)
                tt(sc[:, 4:5], sc[:, 10:11], sc[:, 2:3], op=Alu.subtract)  # rt
                tt(sc[:, 5:6], sc[:, 11:12], sc[:, 3:4], op=Alu.subtract)  # d
                tt(sc[:, 8:9], sc[:, 2:3], sc[:, 0:1], op=Alu.add)  # top + sh
                tt(sc[:, 9:10], sc[:, 3:4], sc[:, 1:2], op=Alu.add)  # left + sw

                # ---- 4. row gather: 768 rows in one dma_gather ----
                gat = wp.tile([P, NJ, GW], f32, tag="gat")
                if GATHER_MODE == "dma_gather":
                    t48 = wp.tile([P, 48], f32, tag="t48")
                    ts(t48[:], ct["c_g_y"], sc[:, 4:5], None, op0=Alu.add)
                    ts(t48[:], t48[:], 0.0, float(H - 1), op0=Alu.max, op1=Alu.min)
                    tt(t48[:], t48[:], ct["c_g_c"], op=Alu.add)
                    ts(t48[:], t48[:], float(i * CH_IN * H), None, op0=Alu.add)
                    ri32 = wp.tile([P, 48], i32, tag="ri32")
                    cpy(ri32[:], t48[:])
                    ri16 = wp.tile([P, 48], i16, tag="ri16")
                    cpy(ri16[:], ri32[:])
                    nc.gpsimd.dma_gather(
                        out_ap=gat[:, : NJ * GW].rearrange("p (j w) -> p j w", j=NJ),
                        in_ap=x[:].rearrange("b c h w -> (b c h) w"),
                        idxs_ap=ri16[:],
                        num_idxs=NJ * P,
                        num_idxs_reg=NJ * P,
                        elem_size=GW,
                    )
                else:
                    t6 = wp.tile([P, NJ], f32, tag="t6")
                    ts(t6[:], ct["c_y_tab"], sc[:, 4:5], None, op0=Alu.add)
                    ts(t6[:], t6[:], 0.0, float(H - 1), op0=Alu.max, op1=Alu.min)
                    tt(t6[:], t6[:], ct["c_c_tab"], op=Alu.add)
                    ts(t6[:], t6[:], float(i * CH_IN * H), None, op0=Alu.add)
                    ri = wp.tile([P, NJ], i32, tag="ri")
                    cpy(ri[:], t6[:])
                    for j in range(NJ):
                        nc.gpsimd.indirect_dma_start(
                            out=gat[:, j, :],
                            out_offset=None,
                            in_=x[:].rearrange("b c h w -> (b c h) w"),
                            in_offset=IndirectOffsetOnAxis(ap=ri[:, j : j + 1], axis=0),
                        )

                # ---- 5. fine column shift via ap_gather ----
                t96 = wp.tile([P, (NJ * S) // 16], f32, tag="t96")
                ts(t96[:], ct["c_jx_x"], sc[:, 5:6], None, op0=Alu.add)
                ts(t96[:], t96[:], 0.0, float(GW - 1), op0=Alu.max, op1=Alu.min)
                tt(t96[:], t96[:], ct["c_jx_j"], op=Alu.add)
                ci32 = wp.tile([P, (NJ * S) // 16], i32, tag="ci32")
                cpy(ci32[:], t96[:])
                ci = wp.tile([P, (NJ * S) // 16], i16, tag="ci")
                cpy(ci[:], ci32[:])
                ext = wp.tile([P, NJ * S], f32, tag="ext")
                nc.gpsimd.ap_gather(
                    ext[:],
                    gat[:].rearrange("p j w -> p (j w)"),
                    ci[:],
                    channels=P,
                    num_elems=NJ * GW,
                    d=1,
                    num_idxs=NJ * S,
                )

                # ---- 6. validity masks + store ----
                mx = wp.tile([P, S], f32, tag="mx")
                txm = wp.tile([P, S], f32, tag="txm")
                ts(mx[:], call_t[:, 0:S], sc[:, 3:4], None, op0=Alu.is_ge)
                ts(txm[:], call_t[:, 0:S], sc[:, 9:10], None, op0=Alu.is_lt)
                tt(mx[:], mx[:], txm[:], op=Alu.mult)
                my = wp.tile([P, NJ], f32, tag="my")
                myb = wp.tile([P, NJ], f32, tag="myb")
                ts(my[:], ct["c_y_tab"], sc[:, 2:3], None, op0=Alu.is_ge)
                ts(myb[:], ct["c_y_tab"], sc[:, 8:9], None, op0=Alu.is_lt)
                tt(my[:], my[:], myb[:], op=Alu.mult)
                fin = wp.tile([P, NJ, S], f32, tag="fin")
                for j in range(NJ):
                    st(
                        fin[:, j, :],
                        ext[:, j * S : (j + 1) * S],
                        my[:, j : j + 1],
                        mx[:],
                        op0=Alu.mult,
                        op1=Alu.mult,
                    )
                nc.sync.dma_start(
                    y[i].rearrange("c (h p) x -> p (c h) x", p=P), fin[:]
                )
                if DEBUG_TAPS:
                    nc.sync.dma_start(d_ar[i], ar[:])
                    nc.sync.dma_start(d_sc[i], sc[:])
                    nc.sync.dma_start(d_ci[i], ci[:])
                    nc.sync.dma_start(d_ext[i], ext[:])
    nc.finalize()
    return nc


_CACHE: dict[str, object] = {}


def kernel(input1: np.ndarray, input2: np.ndarray, **_: np.ndarray) -> np.ndarray:
    input1 = np.ascontiguousarray(np.asarray(input1, dtype=np.float32))
    if "nc" not in _CACHE:
        _CACHE["nc"] = _build()
        _CACHE["consts"] = _consts()
    nc = _CACHE["nc"]
    consts = _CACHE["consts"]
    in_maps = [
        {"x": np.ascontiguousarray(input1[k * BPC : (k + 1) * BPC]), **consts}
        for k in range(N_CORES)
    ]
    res = run_bass_kernel_spmd(nc, in_maps, core_ids=list(range(N_CORES)))
    out = np.concatenate([r["y"] for r in res.results], axis=0)
    return out.astype(np.float32)


if __name__ == "__main__":
    rng = np.random.default_rng(1)
    x = rng.standard_normal((B, CH_IN, H, W), dtype=np.float32)
    print(kernel(x, np.zeros((B, C, S, S), np.float32)).shape)

